# revision 24
# baseline (speedup 1.0000x reference)
"""Trainium2 Bass kernel for modulated conv1d (StyleGAN-style Conv1DMod).

Reference computation (per batch sample b):
  wm[k,c,f]  = kern[k,c,f] * coef * (style[b,c] + 1)        (modulate)
  denom[f]   = rsqrt(sum_{k,c} wm[k,c,f]^2)                 (demodulate)
  out[b,f,w] = denom[f] * sum_{k,c} wm[k,c,f] * feat[b,c,w+k-1]   (SAME conv)

Sharding: data-parallel over batch B=8 -> one sample per NeuronCore.

The conv is 192 PSUM-accumulated bf16 matmuls (1 col/cycle on the PE --
same rate as fp32r but half the DMA/SBUF bytes). The per-channel
modulation scale s1[c] = coef*(style[c]+1) commutes with the conv
contraction, so the host folds it into the bf16 feature cast
(feat*s1[c]) and ships the RAW kernel re-laid-out as one contiguous
[128, K*C/128*128] block per output f-tile: every DMA on this device
costs ~3 us end to end (issue + DGE start delay + transfer + semaphore
propagation), so the first conv group must gate on ONE transfer, not
six. The demodulation denominator is computed on-device as
rsqrt(sum_c (sum_k kern^2)[c,f] * s1[c]^2) with DVE squares/adds and a
tiny fp32 matmul against s1^2, and is folded into the PSUM->SBUF copy
(fp32 -> bf16 output). Warmup matmuls on a zeroed tile bring the PE out
of its low p-state while the first operands are in flight. Feature
chunks (small head chunk first, then few big tiles -- each chunk switch
costs a dead PE slot) stream on the SP HWDGE queue; weights + stores
ride the Activation HWDGE queue; stage tiles are sized so each store
fires as soon as its copies land (Tile deps are tile-granular).
"""

import numpy as np

import concourse.bass as bass
import concourse.mybir as mybir
import concourse.tile as tile

B, C, W, K, F = 8, 256, 8192, 3, 256
COEF = 1.0 / float(np.sqrt(K * C))

P = 128
CT = C // P  # 2 contraction (channel) tiles
FT = F // P  # 2 output-partition tiles
WTILE = 512  # matmul moving-operand width (one PSUM bank of fp32)
# chunk widths: small head chunk so the first matmul group's data is in
# SBUF before the PE finishes its preamble; few big tiles after that
CW = [512, 1536, 6144]
CS = [0, 512, 2048]  # chunk start cols
NJ = len(CW)
# output staging piece widths per chunk (each piece = one stage tile that
# stores as soon as its copies complete)
SPIECE = [512, 1536, 1024]
NWARM = 7  # PE p-state warmup matmuls

MAX_WAITS = 1  # walrus codegen in this container rejects >1 sync wait per inst


def _split_sync_waits(nc, limit=MAX_WAITS):
    """Move excess sem-waits onto NoOps inserted before the offending
    instruction (same engine, program order preserved)."""
    uid = 0
    for fn in nc.m.functions:
        for bb in fn.blocks:
            insts = bb.instructions
            changed = False
            newlist = []
            for ins in insts:
                si = ins.sync_info
                if si is not None and len(si.on_wait) > limit:
                    waits = list(si.on_wait)
                    keep = waits[-limit:]
                    excess = waits[:-limit]
                    for k in range(0, len(excess), limit):
                        nop = mybir.InstNoOp(name=f"waitsplit-{uid}", ins=[], outs=[])
                        uid += 1
                        nop.engine = ins.engine
                        nop.sync_info = mybir.SyncInfo(
                            on_wait=excess[k : k + limit], on_update=[]
                        )
                        newlist.append(nop)
                    ins.sync_info = mybir.SyncInfo(
                        on_wait=keep, on_update=list(si.on_update)
                    )
                    changed = True
                newlist.append(ins)
            if changed:
                bb.instructions = newlist


def _conv1dmod_body(tc, feat, s1p, wblk, out):
    nc = tc.nc
    f32 = mybir.dt.float32
    bf16 = mybir.dt.bfloat16

    with (
        tc.tile_pool(name="xbuf", bufs=1) as xbuf,
        tc.tile_pool(name="wbuf", bufs=1) as wbuf,
        tc.tile_pool(name="stage", bufs=3) as stage_pool,
        tc.tile_pool(name="psum", bufs=8, space="PSUM") as psum_pool,
    ):
        # ---- feature: SBUF-resident per-chunk tiles, streamed in order on
        # the SP HWDGE queue. Persistent tiles (one tag per chunk) mean no
        # WAR hazards: all loads issue back-to-back and the matmul stream
        # chases them chunk by chunk. Each tile holds a 1-col halo on both
        # sides (zero at the edges of W, else re-loaded overlap).
        xb = [
            [
                xbuf.tile([P, CW[j] + 2], bf16, tag=f"x_{ct}_{j}", name=f"x_{ct}_{j}")
                for j in range(NJ)
            ]
            for ct in range(CT)
        ]
        # edge-halo zeros + PE warmup tile on the (otherwise idle) gpsimd
        # engine so the Vector queue is free for the demod chain
        warm = wbuf.tile([P, WTILE], bf16, tag="warm")
        nc.gpsimd.memset(warm[:], 0.0)
        for ct in range(CT):
            nc.gpsimd.memset(xb[ct][0][:, 0:1], 0.0)
            nc.gpsimd.memset(xb[ct][NJ - 1][:, CW[NJ - 1] + 1 : CW[NJ - 1] + 2], 0.0)
        for j in range(NJ):
            lo = CS[j] - 1
            hi = CS[j] + CW[j] + 1
            dst_lo = 0
            if lo < 0:
                dst_lo = 1
                lo = 0
            hi = min(hi, W)
            for ct in range(CT):
                crow = slice(ct * P, (ct + 1) * P)
                nc.sync.dma_start(
                    xb[ct][j][:, dst_lo : dst_lo + (hi - lo)], feat[crow, lo:hi]
                )

        # ---- weights on the Activation HWDGE queue (idle until the stores
        # begin): one [128, CT*K*128] bf16 block per ft. The conv schedule
        # runs all early ft0 groups first, so only the ft0 block gates the
        # stream head; the ft1 block has ~24 matmuls of slack. s1 rides the
        # gpsimd SWDGE queue -- slow, but only the demod reduction needs it.
        wbt = []
        for ft in range(FT):
            t = wbuf.tile([P, CT * K * P], bf16, tag=f"wblk_{ft}")
            nc.scalar.dma_start(t[:], wblk[ft])
            wbt.append(t)
        s1 = wbuf.tile([P, CT], f32, tag="s1")
        with nc.allow_non_contiguous_dma(reason="256-elem style vector"):
            nc.gpsimd.dma_start(s1[:], s1p[:, :])

        def wslice(ct, k, ft):
            a = (ct * K + k) * P
            return wbt[ft][:, a : a + P]

        # ---- PE p-state warmup: harmless matmuls on the zeroed tile keep
        # the Tensor engine ramping while the real operands are in flight.
        wps = psum_pool.tile([P, WTILE], f32, tag="psum")
        for _ in range(NWARM):
            nc.tensor.matmul(wps[:], warm[:, 0:P], warm[:], start=True, stop=True)

        def emit_mms(j, ft):
            """Emit the psum accumulation groups for (chunk j, ft)."""
            pss = []
            for i in range(CW[j] // WTILE):
                ps = psum_pool.tile([P, WTILE], f32, tag="psum")
                first = True
                for ct in range(CT):
                    for k in range(K):
                        base = i * WTILE + k
                        nc.tensor.matmul(
                            ps[:],
                            wslice(ct, k, ft),
                            xb[ct][j][:, base : base + WTILE],
                            start=first,
                            stop=(ct == CT - 1 and k == K - 1),
                        )
                        first = False
                pss.append(ps)
            return pss

        def emit_copies(j, ft, pss):
            """Demodulating PSUM->SBUF copies (fp32 -> bf16), staged in
            SPIECE[j]-wide tiles so each store fires as soon as its copies
            land."""
            out_rows = slice(ft * P, (ft + 1) * P)
            piece = SPIECE[j]
            # the kernel's very last stage pieces go per-group so the final
            # store fires one copy after the final matmul
            terminal = j == NJ - 1 and ft == FT - 1
            pieces = []
            n = len(pss) * WTILE
            h = 0
            while h * WTILE < n:
                p_ = piece
                if terminal and (len(pss) - h) * WTILE <= piece:
                    p_ = WTILE
                pieces.append((h, p_ // WTILE))
                h += p_ // WTILE
            for h, per in pieces:
                pw = per * WTILE
                st = stage_pool.tile([P, pw], bf16, tag=f"st{pw}")
                for i in range(per):
                    nc.vector.tensor_scalar_mul(
                        st[:, i * WTILE : (i + 1) * WTILE],
                        pss[h + i][:],
                        denom[:, ft : ft + 1],
                    )
                lo = CS[j] + h * WTILE
                nc.scalar.dma_start(out[out_rows, lo : lo + pw], st[:])

        # head chunks go ahead of the demod chain so the PE queue has work
        # the moment the weights + chunk0 land; all early ft0 groups first
        # (only wblk[0] gates them), then ft0... ft1 interleave resumes.
        pss00 = emit_mms(0, 0)
        pss10 = emit_mms(1, 0)
        pss01 = emit_mms(0, 1)

        # ---- demodulation scale ----
        # denom[f] = rsqrt(sum_c (sum_k kern[k,c,f]^2) * s1[c]^2), with the
        # c-contraction done by a tiny fp32 matmul against s1^2.
        # bf16 operands for the tiny reduction matmuls: fp32 stationary
        # loads need two half-speed LDWEIGHTS passes on the PE (~1.7 us of
        # stream time); bf16 keeps the denominator well within tolerance.
        # vector-op order matches operand arrival: ft0 squares first (its
        # block lands ~2.5 us before ft1's), s1^2 last (SWDGE latency).
        ksum = [[None] * CT for _ in range(FT)]
        for ft in range(FT):
            sq = wbuf.tile([P, CT * K * P], f32, tag=f"sq_{ft}")
            nc.vector.tensor_mul(sq[:], wbt[ft][:], wbt[ft][:])
            for ct in range(CT):
                tmp = wbuf.tile([P, P], f32, tag=f"ssqt_{ft}_{ct}")
                sst = wbuf.tile([P, P], bf16, tag=f"ssq_{ft}_{ct}")
                a = ct * K * P
                nc.vector.tensor_add(tmp[:], sq[:, a : a + P], sq[:, a + P : a + 2 * P])
                nc.vector.tensor_add(sst[:], tmp[:], sq[:, a + 2 * P : a + 3 * P])
                ksum[ft][ct] = sst
        s1sq = wbuf.tile([P, CT], bf16, tag="s1sq")
        nc.vector.tensor_mul(s1sq[:], s1[:], s1[:])
        # the demod accumulator borrows a psum-pool rotation slot (frees the
        # 8th PSUM bank for the conv pipeline)
        dp = psum_pool.tile([P, WTILE], f32, tag="psum")
        for ft in range(FT):
            for ct in range(CT):
                nc.tensor.matmul(
                    dp[:, ft : ft + 1],
                    ksum[ft][ct][:],
                    s1sq[:, ct : ct + 1],
                    start=(ct == 0),
                    stop=(ct == CT - 1),
                )
        denom = wbuf.tile([P, FT], f32, tag="denom")
        nc.scalar.activation(denom[:], dp[:, 0:FT], mybir.ActivationFunctionType.Sqrt)
        nc.vector.reciprocal(denom[:], denom[:])

        # ---- conv ----
        emit_copies(0, 0, pss00)
        emit_copies(1, 0, pss10)
        emit_copies(0, 1, pss01)
        emit_copies(1, 1, emit_mms(1, 1))
        for j in range(2, NJ):
            for ft in range(FT):
                emit_copies(j, ft, emit_mms(j, ft))


def build_bass():
    nc = bass.Bass(name="conv1dmod")
    feat = nc.dram_tensor("feature", [C, W], mybir.dt.bfloat16, kind="ExternalInput")
    s1p = nc.dram_tensor("s1p", [P, CT], mybir.dt.float32, kind="ExternalInput")
    wblk = nc.dram_tensor(
        "wblk", [FT, P, CT * K * P], mybir.dt.bfloat16, kind="ExternalInput"
    )
    out = nc.dram_tensor("out", [F, W], mybir.dt.bfloat16, kind="ExternalOutput")
    with tile.TileContext(nc) as tc:
        _conv1dmod_body(tc, feat, s1p, wblk, out)
    _split_sync_waits(nc)
    return nc


_NC_CACHE = None


def _prep_inputs(feature, style, kernel):
    """Host-side staging: bf16 casts, modulation folded into the feature,
    kernel re-laid-out as per-ft contiguous stationary blocks."""
    import ml_dtypes

    feature = np.ascontiguousarray(feature, dtype=np.float32)
    style = np.ascontiguousarray(style, dtype=np.float32)
    kernel = np.ascontiguousarray(kernel, dtype=np.float32)
    s1 = (style + 1.0) * COEF  # [B, C]
    feature_m = (feature * s1[:, :, None]).astype(ml_dtypes.bfloat16)
    s1p = np.ascontiguousarray(s1.reshape(B, CT, P).transpose(0, 2, 1))
    # wblk[ft, p, (ct*K + k)*128 + f'] = kern[k, ct*128 + p, ft*128 + f']
    wblk = np.ascontiguousarray(
        kernel.astype(ml_dtypes.bfloat16)
        .reshape(K, CT, P, FT, P)
        .transpose(3, 2, 1, 0, 4)
        .reshape(FT, P, CT * K * P)
    )
    return feature_m, s1p, wblk


def kernel(feature, style, kernel):
    """Full-input entry point: shard over batch across 8 cores, run, gather."""
    global _NC_CACHE
    from concourse.bass_utils import run_bass_kernel_spmd

    if _NC_CACHE is None:
        _NC_CACHE = build_bass()
    nc = _NC_CACHE

    feature_m, s1p, wblk = _prep_inputs(feature, style, kernel)
    in_maps = [
        {"feature": feature_m[b], "s1p": s1p[b], "wblk": wblk} for b in range(B)
    ]
    res = run_bass_kernel_spmd(nc, in_maps, core_ids=list(range(B)))
    return np.stack(
        [r["out"].astype(np.float32) for r in res.results], axis=0
    )


# revision 25
# speedup vs baseline: 1.0003x; 1.0003x over previous
"""Trainium2 Bass kernel for modulated conv1d (StyleGAN-style Conv1DMod).

Reference computation (per batch sample b):
  wm[k,c,f]  = kern[k,c,f] * coef * (style[b,c] + 1)        (modulate)
  denom[f]   = rsqrt(sum_{k,c} wm[k,c,f]^2)                 (demodulate)
  out[b,f,w] = denom[f] * sum_{k,c} wm[k,c,f] * feat[b,c,w+k-1]   (SAME conv)

Sharding: data-parallel over batch B=8 -> one sample per NeuronCore.

The conv is 192 PSUM-accumulated bf16 matmuls (1 col/cycle on the PE --
same rate as fp32r but half the DMA/SBUF bytes). The per-channel
modulation scale s1[c] = coef*(style[c]+1) commutes with the conv
contraction, so the host folds it into the bf16 feature cast
(feat*s1[c]) and ships the RAW kernel re-laid-out as one contiguous
[128, K*C/128*128] block per output f-tile: every DMA on this device
costs ~3 us end to end (issue + DGE start delay + transfer + semaphore
propagation), so the first conv group must gate on ONE transfer, not
six. The demodulation denominator is computed on-device as
rsqrt(sum_c (sum_k kern^2)[c,f] * s1[c]^2) with DVE squares/adds and a
tiny fp32 matmul against s1^2, and is folded into the PSUM->SBUF copy
(fp32 -> bf16 output). Warmup matmuls on a zeroed tile bring the PE out
of its low p-state while the first operands are in flight. Feature
chunks (small head chunk first, then few big tiles -- each chunk switch
costs a dead PE slot) stream on the SP HWDGE queue; weights + stores
ride the Activation HWDGE queue; stage tiles are sized so each store
fires as soon as its copies land (Tile deps are tile-granular).
"""

import numpy as np

import concourse.bass as bass
import concourse.mybir as mybir
import concourse.tile as tile

B, C, W, K, F = 8, 256, 8192, 3, 256
COEF = 1.0 / float(np.sqrt(K * C))

P = 128
CT = C // P  # 2 contraction (channel) tiles
FT = F // P  # 2 output-partition tiles
WTILE = 512  # matmul moving-operand width (one PSUM bank of fp32)
# chunk widths: small head chunk so the first matmul group's data is in
# SBUF before the PE finishes its preamble; few big tiles after that
CW = [512, 1536, 6144]
CS = [0, 512, 2048]  # chunk start cols
NJ = len(CW)
# output staging piece widths per chunk (each piece = one stage tile that
# stores as soon as its copies complete)
SPIECE = [512, 1536, 1024]
NWARM = 7  # PE p-state warmup matmuls

MAX_WAITS = 1  # walrus codegen in this container rejects >1 sync wait per inst


def _split_sync_waits(nc, limit=MAX_WAITS):
    """Move excess sem-waits onto NoOps inserted before the offending
    instruction (same engine, program order preserved)."""
    uid = 0
    for fn in nc.m.functions:
        for bb in fn.blocks:
            insts = bb.instructions
            changed = False
            newlist = []
            for ins in insts:
                si = ins.sync_info
                if si is not None and len(si.on_wait) > limit:
                    waits = list(si.on_wait)
                    keep = waits[-limit:]
                    excess = waits[:-limit]
                    for k in range(0, len(excess), limit):
                        nop = mybir.InstNoOp(name=f"waitsplit-{uid}", ins=[], outs=[])
                        uid += 1
                        nop.engine = ins.engine
                        nop.sync_info = mybir.SyncInfo(
                            on_wait=excess[k : k + limit], on_update=[]
                        )
                        newlist.append(nop)
                    ins.sync_info = mybir.SyncInfo(
                        on_wait=keep, on_update=list(si.on_update)
                    )
                    changed = True
                newlist.append(ins)
            if changed:
                bb.instructions = newlist


def _conv1dmod_body(tc, feat, s1p, wblk, out):
    nc = tc.nc
    f32 = mybir.dt.float32
    bf16 = mybir.dt.bfloat16

    with (
        tc.tile_pool(name="xbuf", bufs=1) as xbuf,
        tc.tile_pool(name="wbuf", bufs=1) as wbuf,
        tc.tile_pool(name="stage", bufs=3) as stage_pool,
        tc.tile_pool(name="psum", bufs=8, space="PSUM") as psum_pool,
    ):
        # ---- feature: SBUF-resident per-chunk tiles, streamed in order on
        # the SP HWDGE queue. Persistent tiles (one tag per chunk) mean no
        # WAR hazards: all loads issue back-to-back and the matmul stream
        # chases them chunk by chunk. Each tile holds a 1-col halo on both
        # sides (zero at the edges of W, else re-loaded overlap).
        xb = [
            [
                xbuf.tile([P, CW[j] + 2], bf16, tag=f"x_{ct}_{j}", name=f"x_{ct}_{j}")
                for j in range(NJ)
            ]
            for ct in range(CT)
        ]
        # edge-halo zeros + PE warmup tile on the (otherwise idle) gpsimd
        # engine so the Vector queue is free for the demod chain
        warm = wbuf.tile([P, WTILE], bf16, tag="warm")
        nc.gpsimd.memset(warm[:], 0.0)
        for ct in range(CT):
            nc.gpsimd.memset(xb[ct][0][:, 0:1], 0.0)
            nc.gpsimd.memset(xb[ct][NJ - 1][:, CW[NJ - 1] + 1 : CW[NJ - 1] + 2], 0.0)
        for j in range(NJ):
            lo = CS[j] - 1
            hi = CS[j] + CW[j] + 1
            dst_lo = 0
            if lo < 0:
                dst_lo = 1
                lo = 0
            hi = min(hi, W)
            for ct in range(CT):
                crow = slice(ct * P, (ct + 1) * P)
                nc.sync.dma_start(
                    xb[ct][j][:, dst_lo : dst_lo + (hi - lo)], feat[crow, lo:hi]
                )

        # ---- weights on the Activation HWDGE queue (idle until the stores
        # begin): one [128, CT*K*128] bf16 block per ft. The conv schedule
        # runs all early ft0 groups first, so only the ft0 block gates the
        # stream head; the ft1 block has ~24 matmuls of slack. s1 rides the
        # gpsimd SWDGE queue -- slow, but only the demod reduction needs it.
        wbt = []
        for ft in range(FT):
            t = wbuf.tile([P, CT * K * P], bf16, tag=f"wblk_{ft}")
            nc.scalar.dma_start(t[:], wblk[ft])
            wbt.append(t)
        s1 = wbuf.tile([P, CT], f32, tag="s1")
        nc.scalar.dma_start(s1[:], s1p[:, :])

        def wslice(ct, k, ft):
            a = (ct * K + k) * P
            return wbt[ft][:, a : a + P]

        # ---- PE p-state warmup: harmless matmuls on the zeroed tile keep
        # the Tensor engine ramping while the real operands are in flight.
        wps = psum_pool.tile([P, WTILE], f32, tag="psum")
        for _ in range(NWARM):
            nc.tensor.matmul(wps[:], warm[:, 0:P], warm[:], start=True, stop=True)

        def emit_mms(j, ft):
            """Emit the psum accumulation groups for (chunk j, ft)."""
            pss = []
            for i in range(CW[j] // WTILE):
                ps = psum_pool.tile([P, WTILE], f32, tag="psum")
                first = True
                for ct in range(CT):
                    for k in range(K):
                        base = i * WTILE + k
                        nc.tensor.matmul(
                            ps[:],
                            wslice(ct, k, ft),
                            xb[ct][j][:, base : base + WTILE],
                            start=first,
                            stop=(ct == CT - 1 and k == K - 1),
                        )
                        first = False
                pss.append(ps)
            return pss

        def emit_copies(j, ft, pss):
            """Demodulating PSUM->SBUF copies (fp32 -> bf16), staged in
            SPIECE[j]-wide tiles so each store fires as soon as its copies
            land."""
            out_rows = slice(ft * P, (ft + 1) * P)
            piece = SPIECE[j]
            # the kernel's very last stage pieces go per-group so the final
            # store fires one copy after the final matmul
            terminal = j == NJ - 1 and ft == FT - 1
            pieces = []
            n = len(pss) * WTILE
            h = 0
            while h * WTILE < n:
                p_ = piece
                if terminal and (len(pss) - h) * WTILE <= piece:
                    p_ = WTILE
                pieces.append((h, p_ // WTILE))
                h += p_ // WTILE
            for h, per in pieces:
                pw = per * WTILE
                st = stage_pool.tile([P, pw], bf16, tag=f"st{pw}")
                for i in range(per):
                    nc.vector.tensor_scalar_mul(
                        st[:, i * WTILE : (i + 1) * WTILE],
                        pss[h + i][:],
                        denom[:, ft : ft + 1],
                    )
                lo = CS[j] + h * WTILE
                nc.scalar.dma_start(out[out_rows, lo : lo + pw], st[:])

        # head chunks go ahead of the demod chain so the PE queue has work
        # the moment the weights + chunk0 land; all early ft0 groups first
        # (only wblk[0] gates them), then ft0... ft1 interleave resumes.
        pss00 = emit_mms(0, 0)
        pss10 = emit_mms(1, 0)
        pss01 = emit_mms(0, 1)

        # ---- demodulation scale ----
        # denom[f] = rsqrt(sum_c (sum_k kern[k,c,f]^2) * s1[c]^2), with the
        # c-contraction done by a tiny fp32 matmul against s1^2.
        # bf16 operands for the tiny reduction matmuls: fp32 stationary
        # loads need two half-speed LDWEIGHTS passes on the PE (~1.7 us of
        # stream time); bf16 keeps the denominator well within tolerance.
        # vector-op order matches operand arrival: ft0 squares first (its
        # block lands ~2.5 us before ft1's), s1^2 last (SWDGE latency).
        ksum = [[None] * CT for _ in range(FT)]
        for ft in range(FT):
            sq = wbuf.tile([P, CT * K * P], f32, tag=f"sq_{ft}")
            nc.vector.tensor_mul(sq[:], wbt[ft][:], wbt[ft][:])
            for ct in range(CT):
                tmp = wbuf.tile([P, P], f32, tag=f"ssqt_{ft}_{ct}")
                sst = wbuf.tile([P, P], bf16, tag=f"ssq_{ft}_{ct}")
                a = ct * K * P
                nc.vector.tensor_add(tmp[:], sq[:, a : a + P], sq[:, a + P : a + 2 * P])
                nc.vector.tensor_add(sst[:], tmp[:], sq[:, a + 2 * P : a + 3 * P])
                ksum[ft][ct] = sst
        s1sq = wbuf.tile([P, CT], bf16, tag="s1sq")
        nc.vector.tensor_mul(s1sq[:], s1[:], s1[:])
        # the demod accumulator borrows a psum-pool rotation slot (frees the
        # 8th PSUM bank for the conv pipeline)
        dp = psum_pool.tile([P, WTILE], f32, tag="psum")
        for ft in range(FT):
            for ct in range(CT):
                nc.tensor.matmul(
                    dp[:, ft : ft + 1],
                    ksum[ft][ct][:],
                    s1sq[:, ct : ct + 1],
                    start=(ct == 0),
                    stop=(ct == CT - 1),
                )
        denom = wbuf.tile([P, FT], f32, tag="denom")
        nc.scalar.activation(denom[:], dp[:, 0:FT], mybir.ActivationFunctionType.Sqrt)
        nc.vector.reciprocal(denom[:], denom[:])

        # ---- conv ----
        emit_copies(0, 0, pss00)
        emit_copies(1, 0, pss10)
        emit_copies(0, 1, pss01)
        emit_copies(1, 1, emit_mms(1, 1))
        for j in range(2, NJ):
            for ft in range(FT):
                emit_copies(j, ft, emit_mms(j, ft))


def build_bass():
    nc = bass.Bass(name="conv1dmod")
    feat = nc.dram_tensor("feature", [C, W], mybir.dt.bfloat16, kind="ExternalInput")
    s1p = nc.dram_tensor("s1p", [P, CT], mybir.dt.float32, kind="ExternalInput")
    wblk = nc.dram_tensor(
        "wblk", [FT, P, CT * K * P], mybir.dt.bfloat16, kind="ExternalInput"
    )
    out = nc.dram_tensor("out", [F, W], mybir.dt.bfloat16, kind="ExternalOutput")
    with tile.TileContext(nc) as tc:
        _conv1dmod_body(tc, feat, s1p, wblk, out)
    _split_sync_waits(nc)
    return nc


_NC_CACHE = None


def _prep_inputs(feature, style, kernel):
    """Host-side staging: bf16 casts, modulation folded into the feature,
    kernel re-laid-out as per-ft contiguous stationary blocks."""
    import ml_dtypes

    feature = np.ascontiguousarray(feature, dtype=np.float32)
    style = np.ascontiguousarray(style, dtype=np.float32)
    kernel = np.ascontiguousarray(kernel, dtype=np.float32)
    s1 = (style + 1.0) * COEF  # [B, C]
    feature_m = (feature * s1[:, :, None]).astype(ml_dtypes.bfloat16)
    s1p = np.ascontiguousarray(s1.reshape(B, CT, P).transpose(0, 2, 1))
    # wblk[ft, p, (ct*K + k)*128 + f'] = kern[k, ct*128 + p, ft*128 + f']
    wblk = np.ascontiguousarray(
        kernel.astype(ml_dtypes.bfloat16)
        .reshape(K, CT, P, FT, P)
        .transpose(3, 2, 1, 0, 4)
        .reshape(FT, P, CT * K * P)
    )
    return feature_m, s1p, wblk


def kernel(feature, style, kernel):
    """Full-input entry point: shard over batch across 8 cores, run, gather."""
    global _NC_CACHE
    from concourse.bass_utils import run_bass_kernel_spmd

    if _NC_CACHE is None:
        _NC_CACHE = build_bass()
    nc = _NC_CACHE

    feature_m, s1p, wblk = _prep_inputs(feature, style, kernel)
    in_maps = [
        {"feature": feature_m[b], "s1p": s1p[b], "wblk": wblk} for b in range(B)
    ]
    res = run_bass_kernel_spmd(nc, in_maps, core_ids=list(range(B)))
    return np.stack(
        [r["out"].astype(np.float32) for r in res.results], axis=0
    )


# revision 26
# speedup vs baseline: 1.0638x; 1.0635x over previous
"""Trainium2 Bass kernel for modulated conv1d (StyleGAN-style Conv1DMod).

Reference computation (per batch sample b):
  wm[k,c,f]  = kern[k,c,f] * coef * (style[b,c] + 1)        (modulate)
  denom[f]   = rsqrt(sum_{k,c} wm[k,c,f]^2)                 (demodulate)
  out[b,f,w] = denom[f] * sum_{k,c} wm[k,c,f] * feat[b,c,w+k-1]   (SAME conv)

Sharding: data-parallel over batch B=8 -> one sample per NeuronCore.

The conv is 192 PSUM-accumulated bf16 matmuls (1 col/cycle on the PE --
same rate as fp32r but half the DMA/SBUF bytes). The per-channel
modulation scale s1[c] = coef*(style[c]+1) commutes with the conv
contraction, so the host folds it into the bf16 feature cast
(feat*s1[c]) and ships the RAW kernel re-laid-out as one contiguous
[128, K*C/128*128] block per output f-tile: every DMA on this device
costs ~3 us end to end (issue + DGE start delay + transfer + semaphore
propagation), so the first conv group must gate on ONE transfer, not
six. The demodulation denominator is computed on-device as
rsqrt(sum_c (sum_k kern^2)[c,f] * s1[c]^2) with DVE squares/adds and a
tiny fp32 matmul against s1^2, and is folded into the PSUM->SBUF copy
(fp32 -> bf16 output). Warmup matmuls on a zeroed tile bring the PE out
of its low p-state while the first operands are in flight. Feature
chunks (small head chunk first, then few big tiles -- each chunk switch
costs a dead PE slot) stream on the SP HWDGE queue; weights + stores
ride the Activation HWDGE queue; stage tiles are sized so each store
fires as soon as its copies land (Tile deps are tile-granular).
"""

import numpy as np

import concourse.bass as bass
import concourse.mybir as mybir
import concourse.tile as tile

B, C, W, K, F = 8, 256, 8192, 3, 256
COEF = 1.0 / float(np.sqrt(K * C))

P = 128
CT = C // P  # 2 contraction (channel) tiles
FT = F // P  # 2 output-partition tiles
WTILE = 512  # matmul moving-operand width (one PSUM bank of fp32)
# chunk widths: every DMA on this device has ~3 us end-to-end latency and
# completions pipeline ~0.65 us apart, so the early chunks are small (each
# matmul group gates on a small, early-completing transfer) and widen once
# the ring is ahead of the PE
CW = [512, 512, 512, 512, 1024, 1024, 2048, 2048]
CS = [0, 512, 1024, 1536, 2048, 3072, 4096, 6144]  # chunk start cols
NJ = len(CW)
# output staging piece widths per chunk (each piece = one stage tile that
# stores as soon as its copies complete)
SPIECE = [512, 512, 512, 512, 1024, 1024, 1024, 1024]
NWARM = 8  # PE p-state warmup matmuls

MAX_WAITS = 1  # walrus codegen in this container rejects >1 sync wait per inst


def _split_sync_waits(nc, limit=MAX_WAITS):
    """Move excess sem-waits onto NoOps inserted before the offending
    instruction (same engine, program order preserved)."""
    uid = 0
    for fn in nc.m.functions:
        for bb in fn.blocks:
            insts = bb.instructions
            changed = False
            newlist = []
            for ins in insts:
                si = ins.sync_info
                if si is not None and len(si.on_wait) > limit:
                    waits = list(si.on_wait)
                    keep = waits[-limit:]
                    excess = waits[:-limit]
                    for k in range(0, len(excess), limit):
                        nop = mybir.InstNoOp(name=f"waitsplit-{uid}", ins=[], outs=[])
                        uid += 1
                        nop.engine = ins.engine
                        nop.sync_info = mybir.SyncInfo(
                            on_wait=excess[k : k + limit], on_update=[]
                        )
                        newlist.append(nop)
                    ins.sync_info = mybir.SyncInfo(
                        on_wait=keep, on_update=list(si.on_update)
                    )
                    changed = True
                newlist.append(ins)
            if changed:
                bb.instructions = newlist


def _conv1dmod_body(tc, feat, s1p, wblk, out):
    nc = tc.nc
    f32 = mybir.dt.float32
    bf16 = mybir.dt.bfloat16

    with (
        tc.tile_pool(name="xbuf", bufs=1) as xbuf,
        tc.tile_pool(name="wbuf", bufs=1) as wbuf,
        tc.tile_pool(name="stage", bufs=3) as stage_pool,
        tc.tile_pool(name="psum", bufs=8, space="PSUM") as psum_pool,
    ):
        # ---- feature: SBUF-resident per-chunk tiles, streamed in order on
        # the SP HWDGE queue. Persistent tiles (one tag per chunk) mean no
        # WAR hazards: all loads issue back-to-back and the matmul stream
        # chases them chunk by chunk. Each tile holds a 1-col halo on both
        # sides (zero at the edges of W, else re-loaded overlap).
        xb = [
            [
                xbuf.tile([P, CW[j] + 2], bf16, tag=f"x_{ct}_{j}", name=f"x_{ct}_{j}")
                for j in range(NJ)
            ]
            for ct in range(CT)
        ]
        # edge-halo zeros + PE warmup tile on the (otherwise idle) gpsimd
        # engine so the Vector queue is free for the demod chain
        warm = wbuf.tile([P, WTILE], bf16, tag="warm")
        nc.gpsimd.memset(warm[:], 0.0)
        for ct in range(CT):
            nc.gpsimd.memset(xb[ct][0][:, 0:1], 0.0)
            nc.gpsimd.memset(xb[ct][NJ - 1][:, CW[NJ - 1] + 1 : CW[NJ - 1] + 2], 0.0)
        for j in range(NJ):
            lo = CS[j] - 1
            hi = CS[j] + CW[j] + 1
            dst_lo = 0
            if lo < 0:
                dst_lo = 1
                lo = 0
            hi = min(hi, W)
            for ct in range(CT):
                crow = slice(ct * P, (ct + 1) * P)
                nc.sync.dma_start(
                    xb[ct][j][:, dst_lo : dst_lo + (hi - lo)], feat[crow, lo:hi]
                )

        # ---- weights on the Activation HWDGE queue (idle until the stores
        # begin): one [128, CT*K*128] bf16 block per ft. The conv schedule
        # runs all early ft0 groups first, so only the ft0 block gates the
        # stream head; the ft1 block has ~24 matmuls of slack. s1 rides the
        # gpsimd SWDGE queue -- slow, but only the demod reduction needs it.
        wbt = []
        for ft in range(FT):
            t = wbuf.tile([P, CT * K * P], bf16, tag=f"wblk_{ft}")
            nc.scalar.dma_start(t[:], wblk[ft])
            wbt.append(t)
        s1 = wbuf.tile([P, CT], f32, tag="s1")
        nc.scalar.dma_start(s1[:], s1p[:, :])

        def wslice(ct, k, ft):
            a = (ct * K + k) * P
            return wbt[ft][:, a : a + P]

        # ---- PE p-state warmup: harmless matmuls on the zeroed tile keep
        # the Tensor engine ramping while the real operands are in flight.
        wps = psum_pool.tile([P, WTILE], f32, tag="psum")
        for _ in range(NWARM):
            nc.tensor.matmul(wps[:], warm[:, 0:P], warm[:], start=True, stop=True)

        def emit_mms(j, ft):
            """Emit the psum accumulation groups for (chunk j, ft)."""
            pss = []
            for i in range(CW[j] // WTILE):
                ps = psum_pool.tile([P, WTILE], f32, tag="psum")
                first = True
                for ct in range(CT):
                    for k in range(K):
                        base = i * WTILE + k
                        nc.tensor.matmul(
                            ps[:],
                            wslice(ct, k, ft),
                            xb[ct][j][:, base : base + WTILE],
                            start=first,
                            stop=(ct == CT - 1 and k == K - 1),
                        )
                        first = False
                pss.append(ps)
            return pss

        def emit_copies(j, ft, pss):
            """Demodulating PSUM->SBUF copies (fp32 -> bf16), staged in
            SPIECE[j]-wide tiles so each store fires as soon as its copies
            land."""
            out_rows = slice(ft * P, (ft + 1) * P)
            piece = SPIECE[j]
            # the kernel's very last stage pieces go per-group so the final
            # store fires one copy after the final matmul
            terminal = j == NJ - 1 and ft == FT - 1
            pieces = []
            n = len(pss) * WTILE
            h = 0
            while h * WTILE < n:
                p_ = piece
                if terminal and (len(pss) - h) * WTILE <= piece:
                    p_ = WTILE
                pieces.append((h, p_ // WTILE))
                h += p_ // WTILE
            for h, per in pieces:
                pw = per * WTILE
                st = stage_pool.tile([P, pw], bf16, tag=f"st{pw}")
                for i in range(per):
                    nc.vector.tensor_scalar_mul(
                        st[:, i * WTILE : (i + 1) * WTILE],
                        pss[h + i][:],
                        denom[:, ft : ft + 1],
                    )
                lo = CS[j] + h * WTILE
                nc.scalar.dma_start(out[out_rows, lo : lo + pw], st[:])

        # head chunks go ahead of the demod chain so the PE queue has work
        # the moment the weights + chunk0 land; ft0 groups lead (only
        # wblk[0] gates them, wblk[1] lands ~2.5 us later).
        pss00 = emit_mms(0, 0)
        pss10 = emit_mms(1, 0)
        pss01 = emit_mms(0, 1)
        pss11 = emit_mms(1, 1)

        # ---- demodulation scale ----
        # denom[f] = rsqrt(sum_c (sum_k kern[k,c,f]^2) * s1[c]^2), with the
        # c-contraction done by a tiny fp32 matmul against s1^2.
        # bf16 operands for the tiny reduction matmuls: fp32 stationary
        # loads need two half-speed LDWEIGHTS passes on the PE (~1.7 us of
        # stream time); bf16 keeps the denominator well within tolerance.
        # vector-op order matches operand arrival: ft0 squares first (its
        # block lands ~2.5 us before ft1's), s1^2 last (SWDGE latency).
        ksum = [[None] * CT for _ in range(FT)]
        for ft in range(FT):
            sq = wbuf.tile([P, CT * K * P], f32, tag=f"sq_{ft}")
            nc.vector.tensor_mul(sq[:], wbt[ft][:], wbt[ft][:])
            for ct in range(CT):
                tmp = wbuf.tile([P, P], f32, tag=f"ssqt_{ft}_{ct}")
                sst = wbuf.tile([P, P], bf16, tag=f"ssq_{ft}_{ct}")
                a = ct * K * P
                nc.vector.tensor_add(tmp[:], sq[:, a : a + P], sq[:, a + P : a + 2 * P])
                nc.vector.tensor_add(sst[:], tmp[:], sq[:, a + 2 * P : a + 3 * P])
                ksum[ft][ct] = sst
        s1sq = wbuf.tile([P, CT], bf16, tag="s1sq")
        nc.vector.tensor_mul(s1sq[:], s1[:], s1[:])
        # the demod accumulator borrows a psum-pool rotation slot (frees the
        # 8th PSUM bank for the conv pipeline)
        dp = psum_pool.tile([P, WTILE], f32, tag="psum")
        for ft in range(FT):
            for ct in range(CT):
                nc.tensor.matmul(
                    dp[:, ft : ft + 1],
                    ksum[ft][ct][:],
                    s1sq[:, ct : ct + 1],
                    start=(ct == 0),
                    stop=(ct == CT - 1),
                )
        denom = wbuf.tile([P, FT], f32, tag="denom")
        nc.scalar.activation(denom[:], dp[:, 0:FT], mybir.ActivationFunctionType.Sqrt)
        nc.vector.reciprocal(denom[:], denom[:])

        # ---- conv ----
        emit_copies(0, 0, pss00)
        emit_copies(1, 0, pss10)
        emit_copies(0, 1, pss01)
        emit_copies(1, 1, pss11)
        for j in range(2, NJ):
            for ft in range(FT):
                emit_copies(j, ft, emit_mms(j, ft))


def build_bass():
    nc = bass.Bass(name="conv1dmod")
    feat = nc.dram_tensor("feature", [C, W], mybir.dt.bfloat16, kind="ExternalInput")
    s1p = nc.dram_tensor("s1p", [P, CT], mybir.dt.float32, kind="ExternalInput")
    wblk = nc.dram_tensor(
        "wblk", [FT, P, CT * K * P], mybir.dt.bfloat16, kind="ExternalInput"
    )
    out = nc.dram_tensor("out", [F, W], mybir.dt.bfloat16, kind="ExternalOutput")
    with tile.TileContext(nc) as tc:
        _conv1dmod_body(tc, feat, s1p, wblk, out)
    _split_sync_waits(nc)
    return nc


_NC_CACHE = None


def _prep_inputs(feature, style, kernel):
    """Host-side staging: bf16 casts, modulation folded into the feature,
    kernel re-laid-out as per-ft contiguous stationary blocks."""
    import ml_dtypes

    feature = np.ascontiguousarray(feature, dtype=np.float32)
    style = np.ascontiguousarray(style, dtype=np.float32)
    kernel = np.ascontiguousarray(kernel, dtype=np.float32)
    s1 = (style + 1.0) * COEF  # [B, C]
    feature_m = (feature * s1[:, :, None]).astype(ml_dtypes.bfloat16)
    s1p = np.ascontiguousarray(s1.reshape(B, CT, P).transpose(0, 2, 1))
    # wblk[ft, p, (ct*K + k)*128 + f'] = kern[k, ct*128 + p, ft*128 + f']
    wblk = np.ascontiguousarray(
        kernel.astype(ml_dtypes.bfloat16)
        .reshape(K, CT, P, FT, P)
        .transpose(3, 2, 1, 0, 4)
        .reshape(FT, P, CT * K * P)
    )
    return feature_m, s1p, wblk


def kernel(feature, style, kernel):
    """Full-input entry point: shard over batch across 8 cores, run, gather."""
    global _NC_CACHE
    from concourse.bass_utils import run_bass_kernel_spmd

    if _NC_CACHE is None:
        _NC_CACHE = build_bass()
    nc = _NC_CACHE

    feature_m, s1p, wblk = _prep_inputs(feature, style, kernel)
    in_maps = [
        {"feature": feature_m[b], "s1p": s1p[b], "wblk": wblk} for b in range(B)
    ]
    res = run_bass_kernel_spmd(nc, in_maps, core_ids=list(range(B)))
    return np.stack(
        [r["out"].astype(np.float32) for r in res.results], axis=0
    )
